# revision 10
# baseline (speedup 1.0000x reference)
"""Conv2D 3x3 (NCHW, OIHW, stride 1, pad 1) on 8 Trainium2 NeuronCores.

Problem shape: input (32, 128, 56, 56) fp32, weights (256, 128, 3, 3) fp32,
output (32, 256, 56, 56) fp32.

Strategy — 1D Winograd F(4,3) along the width axis (1/2 the direct MACs):
  - Data-parallel over batch: 4 images per core, weights replicated.
  - Host precomputes the Winograd input transform: for each padded row and
    4-wide output tile t, the 6 transform planes (B^T d with points
    {0,±1,±2,∞}), giving V[ci, k, 58 rows, 14 tiles] (fp16). Weights become
    U[dy,k][ci,co] = rows of G @ g (fp16).
  - Device: per (image, 28-row group, co-half) accumulate in PSUM
        m_k = sum_dy U[dy,k].T @ V[k][rows+dy]     (6 planes, 18 matmuls)
    with moving dim 392 = 28 rows x 14 tiles. Each plane is its own
    1-bank PSUM tile (bufs=8): planes free individually after their
    drain copy, so the next group's matmuls pipeline into freed banks
    with no inter-group stall, and the 392-cycle matmuls fully hide the
    ~97ns per-matmul LDWEIGHTS.
  - The raw m-planes are drained PSUM->SBUF as fp16 (copies split across
    ACT/DVE/GpSimd; GpSimd is avoided for the final group so the kernel
    tail never waits on the slow engine) and shipped as one DMA per
    group; the host applies the output transform y = A^T m (coeffs
    1,2,4,8) and interleaves tiles into NCHW fp32.
  - Scheduling notes (inherited from the F(2,3) predecessor): out-DMAs
    keep their own sequencer (sync ring) so they never block the
    PSUM-release chain; boot-time DMA bandwidth is scarce, so image 0's
    V plane heads and the first co-half's weight pieces are interleaved
    fine-grained across both HWDGE rings in first-use order, with 10
    warmup matmuls bridging sequencer boot until the first operands
    land. Later images prefetch on the scalar ring mid-image.
"""

import sys

sys.path.insert(0, "/opt/trn_rl_repo")

import numpy as np

N_CORES = 8
N_FULL = 32
IMGS = N_FULL // N_CORES  # images per core
CIN = 128
COUT = 256
H = W = 56
HP = 58  # padded rows
T = 14  # winograd F(4,3) tiles per row
NK = 6  # transform planes
PLANE = HP * T  # 812 elements per transform plane
GR = 28  # output rows per group
M = GR * T  # 392 moving dim
GROUPS = 2  # row groups per image (2 x 28 = 56)
OUT_LEN = GROUPS * NK * M  # 4704 fp16 per partition per (img, half)

_CACHE = {}

# F(4,3) transform matrices (correlation form, points {0, ±1, ±2, ∞})
_AT = np.array(
    [[1, 1, 1, 1, 1, 0], [0, 1, -1, 2, -2, 0], [0, 1, 1, 4, 4, 0], [0, 1, -1, 8, -8, 1]],
    np.float32,
)
_G = np.array(
    [
        [1 / 4, 0, 0],
        [-1 / 6, -1 / 6, -1 / 6],
        [-1 / 6, 1 / 6, -1 / 6],
        [1 / 24, 1 / 12, 1 / 6],
        [1 / 24, -1 / 12, 1 / 6],
        [0, 0, 1],
    ],
    np.float32,
)
_BT = np.array(
    [
        [4, 0, -5, 0, 1, 0],
        [0, -4, -4, 1, 1, 0],
        [0, 4, -4, -1, 1, 0],
        [0, -2, -1, 2, 1, 0],
        [0, 2, -1, -2, 1, 0],
        [0, 4, 0, -5, 0, 1],
    ],
    np.float32,
)


def _split_sync_waits(nc, mybir, max_waits=1):
    """The walrus build in this container rejects instructions carrying
    more than one semaphore wait; hoist extras onto preceding NOPs on the
    same engine (engine executes them in order, semantics preserved)."""
    ctr = 0
    for f in nc.m.functions:
        for bb in f.blocks:
            new_insts = []
            for ins in bb.instructions:
                si = getattr(ins, "sync_info", None)
                if si is not None and si.on_wait and len(si.on_wait) > max_waits:
                    waits = list(si.on_wait)
                    extra, keep = waits[:-max_waits], waits[-max_waits:]
                    for i in range(0, len(extra), max_waits):
                        ctr += 1
                        nop = mybir.InstNoOp(
                            name=f"{ins.name}_wsplit{ctr}",
                            engine=ins.engine,
                            sync_info=mybir.SyncInfo(
                                on_wait=extra[i : i + max_waits], on_update=[]
                            ),
                            bass_nofuse=True,
                        )
                        new_insts.append(nop)
                    si.on_wait = keep
                new_insts.append(ins)
            bb.instructions[:] = new_insts
    return ctr


def _build():
    import concourse.bass as bass
    import concourse.mybir as mybir
    import concourse.tile as tile

    f32 = mybir.dt.float32
    f16 = mybir.dt.float16

    nc = bass.Bass()
    x = nc.declare_dram_parameter("x", [IMGS, CIN, NK * PLANE], f16, isOutput=False)
    w = nc.declare_dram_parameter("w", [CIN, 2 * NK * 3 * 128], f16, isOutput=False)
    # out[n, half, co, group*2352 + par*1176 + idx*392 + (r_local*14 + t)]
    # fp16 m-planes, plane k stored at (par, idx) = (k%2, k//2) so each
    # drain engine's planes ship contiguously on its own DMA ring
    out = nc.declare_dram_parameter("out", [IMGS, 2, 128, OUT_LEN], f16, isOutput=True)

    x3 = x.rearrange("n p (k e) -> n p k e", k=NK)
    out5 = out.rearrange("n h p (g par idx e) -> n h p g par idx e", g=GROUPS, par=2, idx=3)

    with tile.TileContext(nc) as tc:
        with (
            tc.tile_pool(name="wpool", bufs=1) as wpool,
            tc.tile_pool(name="vpool", bufs=2) as vpool,
            tc.tile_pool(name="opool", bufs=4) as opool,
            tc.tile_pool(name="psum", bufs=8, space="PSUM") as pspool,
        ):
            # Short PE warmup bridges the gap until the first input DMA
            # lands (the HAM window runs the clocks at ~1.2 GHz for the
            # first ~15.7us regardless).
            warm = wpool.tile([128, 256], f16, name="warm")
            nc.vector.memzero(warm[:])
            wps = pspool.tile([128, M], f32, name="ps")
            for _ in range(2):
                nc.tensor.matmul(
                    wps[:, 0:256], lhsT=warm[:, 0:128], rhs=warm[:], start=True, stop=True
                )

            wt = wpool.tile([CIN, 2 * NK * 3 * 128], f16)

            def uslice(h, k, dy):
                c0 = ((h * NK + k) * 3 + dy) * 128
                return wt[:, c0 : c0 + 128]

            HEAD = 30 * T  # rows 0-29 cover group 0 (dy reach 0..29)

            def emit_v_dmas(n, vt, vt3):
                if n == 0:
                    # DMA data starts flowing only ~8.6us in (queue boot),
                    # at ~90-150 GB/s per ring: spread first-use pieces
                    # over three rings in strict consumption order. Sync's
                    # hardware ring starts earliest and is fastest: it
                    # gets the h0 weight stream; V plane heads ride
                    # scalar (even k) / gpsimd (odd k).
                    for k in range(NK):
                        wc0 = k * 384
                        nc.sync.dma_start(out=wt[:, wc0 : wc0 + 384], in_=w[:, wc0 : wc0 + 384])
                    nc.scalar.dma_start(out=vt3[:, 0, 0 : 16 * T], in_=x3[n, :, 0, 0 : 16 * T])
                    nc.scalar.dma_start(out=vt3[:, 0, 16 * T : HEAD], in_=x3[n, :, 0, 16 * T : HEAD])
                    nc.gpsimd.dma_start(out=vt3[:, 1, 0:HEAD], in_=x3[n, :, 1, 0:HEAD])
                    nc.scalar.dma_start(out=vt3[:, 2, 0:HEAD], in_=x3[n, :, 2, 0:HEAD])
                    nc.gpsimd.dma_start(out=vt3[:, 3, 0:HEAD], in_=x3[n, :, 3, 0:HEAD])
                    nc.scalar.dma_start(out=vt3[:, 4, 0:HEAD], in_=x3[n, :, 4, 0:HEAD])
                    nc.gpsimd.dma_start(out=vt3[:, 5, 0:HEAD], in_=x3[n, :, 5, 0:HEAD])
                    # second co-half weights in k-pair pieces across rings
                    nc.sync.dma_start(out=wt[:, 2304:3072], in_=w[:, 2304:3072])
                    nc.scalar.dma_start(out=wt[:, 3072:3840], in_=w[:, 3072:3840])
                    nc.gpsimd.dma_start(out=wt[:, 3840:4608], in_=w[:, 3840:4608])
                    # rows 30-57 per plane (group 1)
                    for k in range(NK):
                        ring = (nc.scalar, nc.sync)[k % 2]
                        ring.dma_start(
                            out=vt3[:, k, HEAD:PLANE], in_=x3[n, :, k, HEAD:PLANE]
                        )
                else:
                    # later images prefetch as one whole-image DMA
                    # (9.7KB/partition contiguous descriptors); mid-kernel
                    # slack is ~12us so a single end-of-transfer semaphore
                    # is fine and costs scalar only one issue slot
                    nc.scalar.dma_start(out=vt[:], in_=x[n, :, :])

            vt = vpool.tile([CIN, NK * PLANE], f16)
            vt3 = vt.rearrange("p (k e) -> p k e", k=NK)
            emit_v_dmas(0, vt, vt3)

            for n in range(IMGS):
                for g in range(GROUPS):
                    for h in range(2):
                        # image 0's very first unit is split into two
                        # 14-row subgroups so the first matmul's operand
                        # footprint (one weight piece + 16 V rows) clears
                        # the just-booted DMA rings ~3.5us earlier
                        if n == 0 and g == 0 and h == 0:
                            sub = ((0, 14), (14, 14))
                        else:
                            sub = ((g * GR, GR),)
                        final = n == IMGS - 1 and g == GROUPS - 1 and h == 1
                        for r0, rows in sub:
                            MM = rows * T
                            pss = [
                                pspool.tile([128, MM], f32, name="ps")
                                for _ in range(NK)
                            ]
                            for k in range(NK):
                                for dy in range(3):
                                    nc.tensor.matmul(
                                        pss[k][:],
                                        lhsT=uslice(h, k, dy),
                                        rhs=vt3[:, k, (r0 + dy) * T : (r0 + dy + rows) * T],
                                        start=(dy == 0),
                                        stop=(dy == 2),
                                    )
                            # drain raw m-planes PSUM -> SBUF fp16: even k
                            # on ACT, odd k on DVE (GpSimd cannot read
                            # PSUM), each parity contiguous in staging
                            yy = opool.tile([128, NK * MM], f16, name="yy")
                            yyr = yy.rearrange("p (s e) -> p s e", s=NK)
                            for k in range(NK):
                                par, idx = k % 2, k // 2
                                dst = yyr[:, par * 3 + idx, :]
                                if par == 0:
                                    nc.scalar.copy(out=dst, in_=pss[k][:])
                                else:
                                    nc.vector.tensor_copy(out=dst, in_=pss[k][:])
                            # each parity ships on the ring of its drain
                            # engine (waits only same-engine copies, and
                            # splits out-traffic across both rings)
                            lo, hi = r0 - g * GR, r0 - g * GR + rows
                            d5 = out5[n, h, :, g, :, :, lo * T : hi * T]
                            if final:
                                # finer split: the kernel tail waits on a
                                # single-plane 50KB transfer
                                nc.scalar.dma_start(out=d5[:, 0, 0:2], in_=yyr[:, 0:2])
                                nc.scalar.dma_start(out=d5[:, 0, 2], in_=yyr[:, 2])
                                nc.sync.dma_start(out=d5[:, 1, 0:2], in_=yyr[:, 3:5])
                                nc.sync.dma_start(out=d5[:, 1, 2], in_=yyr[:, 5])
                            else:
                                nc.scalar.dma_start(out=d5[:, 0], in_=yyr[:, 0:3])
                                nc.sync.dma_start(out=d5[:, 1], in_=yyr[:, 3:6])
                    # hoist next image's V DMA issue to mid-image so the
                    # transfer completes before that image starts
                    if g == 0 and n + 1 < IMGS:
                        vt_next = vpool.tile([CIN, NK * PLANE], f16)
                        vt3_next = vt_next.rearrange("p (k e) -> p k e", k=NK)
                        emit_v_dmas(n + 1, vt_next, vt3_next)
                if n + 1 < IMGS:
                    vt3 = vt3_next

    _split_sync_waits(nc, mybir)
    return nc


def _prep_inputs(input_batch, weights):
    xf = np.asarray(input_batch, dtype=np.float32)
    xp = np.zeros((N_FULL, CIN, HP, HP), dtype=np.float32)
    xp[:, :, 1:-1, 1:-1] = xf
    # width tiles: cols 4t+c, c=0..5, t=0..13
    D = np.stack([xp[..., c::4][..., :T] for c in range(6)], axis=-1)  # [N,C,58,14,6]
    V = np.einsum("kc,nzrtc->nzkrt", _BT, D).astype(np.float16)
    V = np.ascontiguousarray(V.reshape(N_FULL, CIN, NK * PLANE))

    wf = np.asarray(weights, dtype=np.float32)
    U = np.einsum("ks,ozds->dkoz", _G, wf)  # [3, 6, COUT, CIN]
    # w[ci, ((h*6 + k)*3 + dy)*128 + co] = U[dy, k, h*128 + co, ci]
    wt = np.ascontiguousarray(
        U.reshape(3, NK, 2, 128, CIN)
        .transpose(4, 2, 1, 0, 3)  # [ci, h, k, dy, co]
        .reshape(CIN, 2 * NK * 3 * 128)
        .astype(np.float16)
    )

    in_maps = []
    for i in range(N_CORES):
        in_maps.append(
            {
                "x": np.ascontiguousarray(V[i * IMGS : (i + 1) * IMGS]),
                "w": wt,
            }
        )
    return in_maps


def _assemble(outs):
    # outs: list of [IMGS, 2, 128, OUT_LEN] fp16 per core; layout
    # [n, half, co, group, parity, idx, r_local, t] with k = 2*idx + parity
    full = np.concatenate(outs, axis=0).reshape(N_FULL, 2, 128, GROUPS, 2, 3, GR, T)
    m = full.astype(np.float32)
    # host output transform y = A^T m (coeffs 1,2,4,8)
    m0, m1, m2, m3, m4, m5 = (m[:, :, :, :, k % 2, k // 2] for k in range(NK))
    s, d = m1 + m2, m1 - m2
    p, q = m3 + m4, m3 - m4
    y = np.empty((N_FULL, 2, 128, GROUPS, GR, T, 4), np.float32)
    y[..., 0] = m0 + s + p
    y[..., 1] = d + 2 * q
    y[..., 2] = s + 4 * p
    y[..., 3] = d + 8 * q + m5
    # [n, h, co, g, r, t, j] -> [n, (h co), (g r), (t j)]
    return np.ascontiguousarray(y.reshape(N_FULL, COUT, H, W))


def _run(input_batch, weights, trace=False):
    from concourse.bass_utils import run_bass_kernel_spmd

    if "nc" not in _CACHE:
        _CACHE["nc"] = _build()
    nc = _CACHE["nc"]
    in_maps = _prep_inputs(np.asarray(input_batch), np.asarray(weights))
    res = run_bass_kernel_spmd(nc, in_maps, list(range(N_CORES)), trace=trace)
    outs = [res.results[i]["out"] for i in range(N_CORES)]
    return _assemble(outs), res


def kernel(input_batch, weights):
    full, _ = _run(input_batch, weights, trace=False)
    return full
